# revision 66
# baseline (speedup 1.0000x reference)
"""Trainium2 Bass kernel for a dense transformer decoder block, 8-core SPMD.

Sharding: sequence-parallel. Core c owns token rows [512c:512c+512) of one
batch (GRP=4 cores per batch). Each core computes QKV for its own rows; K and
V shards are AllGathered within the 4-core group; attention runs dense over
all keys with a host-supplied 0/1 mask; wo / MLP are row-local.

Key scheduling/precision choices (v2):
 - K/V are stored and gathered in fp8-e4m3 (halves collective bytes); the
   gathers are chunked into 4 head-quarters (K+V for 4 heads per AllGather)
   so attention can start as soon as the first quarter lands.
 - Projections and the MLP stay bf16 (fp8 projections / fp8 MLP measurably
   break the 2e-2 error budget). Scores run fp8xfp8 (storage-only fp8 of
   q/k is numerically free); probs are e5m2 (huge dynamic range, no offset
   or flush-to-zero hazard); AV and wo run fp8 DoubleRow (2 contraction
   rows per PE cell = half the matmul instructions).
 - 1/sqrt(HD) is applied inside the Exp activation, not folded into wq,
   keeping q at unit scale for fp8 storage.
 - x2 (post-attention residual) stays in SBUF; no DRAM bounce.

Matmuls accumulate in fp32 PSUM; softmax + residual stay fp32.
"""
import math
from contextlib import ExitStack

import numpy as np
import ml_dtypes

import concourse.bacc as bacc
import concourse.bass as bass
import concourse.tile as tile
import concourse.mybir as mybir
from concourse.bass_utils import run_bass_kernel_spmd
from concourse.masks import make_identity

AF = mybir.ActivationFunctionType
BF = mybir.dt.bfloat16
F32 = mybir.dt.float32
F8 = mybir.dt.float8e4
F8E5 = mybir.dt.float8e5
DR = mybir.MatmulPerfMode.DoubleRow

N_CORES = 8
P = 128
B, S, D, H, HD, DFF = 2, 2048, 2048, 16, 128, 8192
GRP = 4                   # cores per batch (AllGather subgroup size)
C = S // GRP              # 512 tokens per core, contiguous rows of one batch
TOK = C
NT = TOK // P             # 4 token tiles
KD = D // P               # 16 contraction tiles over D
NDF = DFF // P            # 64 dff tiles
NC_ = D // 512            # 4 output 512-chunks
NKV = S // P              # 16 kv tiles over the core's batch
NQ = 4                    # AllGather quarters
HPQ = H // NQ             # 4 heads per quarter
KQE = P * HPQ * TOK       # fp8 elements in the K part of a quarter
VQE = NT * P * 512        # fp8 elements in the V part of a quarter
QE = KQE + VQE
EPS = 1e-8
INV_SQRT_HD = 1.0 / math.sqrt(HD)

RG = [[0, 1, 2, 3], [4, 5, 6, 7]]


def _emit(nc):
    x_in = nc.dram_tensor("x_own", [TOK, D], F32, kind="ExternalInput")
    wq_in = nc.dram_tensor("wqt", [H, P, KD, HD], BF, kind="ExternalInput")
    wk_in = nc.dram_tensor("wkt", [H, P, KD, HD], BF, kind="ExternalInput")
    wv_in = nc.dram_tensor("wvt", [NQ, P, KD, 512], BF, kind="ExternalInput")
    wo_in = nc.dram_tensor("wot", [NC_, P, H // 2, 2, 512], F8,
                           kind="ExternalInput")
    f1_in = nc.dram_tensor("fc1t", [NDF, P, KD, HD], BF, kind="ExternalInput")
    f2_in = nc.dram_tensor("fc2t", [NC_, 4, P, 16, 512], BF,
                           kind="ExternalInput")
    f1b_in = nc.dram_tensor("fc1b", [P, NDF], F32, kind="ExternalInput")
    f2b_in = nc.dram_tensor("fc2b", [D], F32, kind="ExternalInput")
    msk_in = nc.dram_tensor("mask", [P, NKV, C], F8E5, kind="ExternalInput")
    y_out = nc.dram_tensor("y", [TOK, D], F32, kind="ExternalOutput")

    xv = x_in.ap().rearrange("(t p) d -> p t d", p=P)     # [P, NT, D] DRAM view
    yv = y_out.ap().rearrange("(t p) d -> p t d", p=P)

    with tile.TileContext(nc) as tc, ExitStack() as ctx:
        singles = ctx.enter_context(tc.tile_pool(name="singles", bufs=1))
        persist = ctx.enter_context(tc.tile_pool(name="persist", bufs=1))
        scratch = ctx.enter_context(tc.tile_pool(name="scratch", bufs=2))
        dram = ctx.enter_context(tc.tile_pool(name="dram", bufs=1, space="DRAM"))

        ident = singles.tile([P, P], BF, tag="ident")
        make_identity(nc, ident)
        ones_row = singles.tile([1, P], F32, tag="onesr")
        nc.vector.memset(ones_row, 1.0)
        # [128, 2, 1]-sliceable all-ones fp8 for DoubleRow denominator mms;
        # the middle dim needs a >=16B stride, hence the padded free dim
        ones8 = singles.tile([P, 2, 16], F8, tag="ones8")
        nc.vector.memset(ones8, 1.0)
        f1b_sb = singles.tile([P, NDF], F32, tag="f1b")
        nc.scalar.dma_start(f1b_sb, f1b_in[:])

        BN_STATS_DIM = nc.vector.BN_STATS_DIM
        BN_AGGR_DIM = nc.vector.BN_AGGR_DIM
        NSUB = D // nc.vector.BN_STATS_FMAX

        def rms_norm_tile(xt):
            """[P, D] fp32 -> [P, 1] fp32 reciprocal of (sqrt(mean(x^2))+eps)."""
            stats = scratch.tile([P, NSUB, BN_STATS_DIM], F32, tag="bst",
                                 name="stats")
            x4 = xt.rearrange("p (s f) -> p s f", s=NSUB)
            for sg in range(NSUB):
                nc.vector.bn_stats(out=stats[:, sg], in_=x4[:, sg])
            mv = scratch.tile([P, BN_AGGR_DIM], F32, tag="bag", name="mv")
            nc.vector.bn_aggr(out=mv, in_=stats)
            msq = scratch.tile([P, 1], F32, tag="msq", name="msq")
            nc.vector.tensor_mul(msq, mv[:, 0:1], mv[:, 0:1])
            nc.vector.tensor_add(msq, msq, mv[:, 1:2])   # mean(x^2)
            lnv = scratch.tile([P, 1], F32, tag="lnv", name="lnv")
            nc.scalar.activation(out=lnv, in_=msq, func=AF.Ln)
            rms = scratch.tile([P, 1], F32, tag="rms", name="rms")
            nc.scalar.activation(out=rms, in_=lnv, func=AF.Exp, scale=0.5)
            nc.vector.tensor_scalar_add(rms, rms, EPS)
            rinv = scratch.tile([P, 1], F32, tag="rinv", name="rinv")
            nc.vector.reciprocal(rinv, rms)
            return rinv

        def transpose_into(nT, nbf, mt, pp):
            for kt in range(KD):
                ps = pp.tile([P, P], BF, tag="tp", name="tps")
                nc.tensor.transpose(ps, nbf[:, kt * P:(kt + 1) * P], ident)
                nc.vector.tensor_copy(
                    out=nT[:, kt, mt * P:(mt + 1) * P], in_=ps)

        # DRAM staging for the collectives (1-D tiles + shaped views)
        kvin = [dram.tile([QE], F8, tag=f"kvi{i}", name=f"kvi{i}")
                for i in range(NQ)]
        kvout = [dram.tile([GRP * QE], F8, tag=f"kvo{i}", name=f"kvo{i}")
                 for i in range(NQ)]
        # sender views; both parts partition-major so every DMA moves 2KB
        # contiguous lines per partition
        kin_k = [t[:][0:KQE].rearrange("(p l t) -> p l t", p=P, l=HPQ)
                 for t in kvin]
        kin_v = [t[:][KQE:QE].rearrange("(p m c) -> p m c", p=P, m=NT)
                 for t in kvin]
        # receiver views: per-rank slices of the r-major gathered quarter
        def kout_k(i, r):
            return kvout[i][:][r * QE:r * QE + KQE].rearrange(
                "(p l t) -> p l t", p=P, l=HPQ)

        def kout_v(i, r):
            return kvout[i][:][r * QE + KQE:(r + 1) * QE].rearrange(
                "(p m c) -> p m c", p=P, m=NT)

        # q / attention-out tiles live until the end of the wo phase
        qa = ctx.enter_context(tc.tile_pool(name="qa", bufs=1))
        qt = qa.tile([P, H, TOK], F8, tag="qt", name="qt")
        avt = qa.tile([P, H, TOK], F8, tag="avt", name="avt")

        # x-tile pool: only needed until the end of phase 1 norms
        xtp_ctx = ExitStack()
        xtp = xtp_ctx.enter_context(tc.tile_pool(name="xtp", bufs=2))

        # mask for attention: own pool so it can be freed after attention
        msk_ctx = ExitStack()
        mskp = msk_ctx.enter_context(tc.tile_pool(name="mskp", bufs=1))
        msk = mskp.tile([P, NKV, C], F8E5, tag="msk", name="msk")

        # gathered K/V, fp8, fully resident during attention; r-major so the
        # per-rank gather DMAs are contiguous 2KB lines per partition
        kv_ctx = ExitStack()
        kvp = kv_ctx.enter_context(tc.tile_pool(name="kvp", bufs=1))
        katt = [kvp.tile([P, GRP, HPQ, TOK], F8, tag=f"ka{i}", name=f"ka{i}")
                for i in range(NQ)]
        vall = [kvp.tile([P, GRP, NT, 512], F8, tag=f"va{i}", name=f"va{i}")
                for i in range(NQ)]

        # ---- phase 1: norm1 + quarter-chunked K/V/Q + AllGathers ----
        with tc.tile_pool(name="pp", bufs=4, space="PSUM") as pp, \
             tc.tile_pool(name="wv_pool", bufs=2) as wv_pool, \
             tc.tile_pool(name="wqk_pool", bufs=4) as wqk_pool, \
             tc.tile_pool(name="kst_pool", bufs=2) as kst_pool, \
             tc.tile_pool(name="vst_pool", bufs=2) as vst_pool:
            n1T = persist.tile([P, KD, TOK], BF, tag="nT", name="nT")
            for mt in range(NT):
                xt = xtp.tile([P, D], F32, tag="xt", name="xt")
                # quarters alternating queues: tile 0 lands ~2x sooner on
                # the cold DMA rings, so norm1 + transposes start earlier
                for q in range(4):
                    eng = nc.sync if q % 2 == 0 else nc.scalar
                    eng.dma_start(xt[:, q * 512:(q + 1) * 512],
                                  xv[:, mt, q * 512:(q + 1) * 512])
                rinv = rms_norm_tile(xt)
                nbf = scratch.tile([P, D], BF, tag="nbf", name="nbf")
                nc.vector.tensor_scalar_mul(nbf, xt, rinv)
                transpose_into(n1T, nbf, mt, pp)

            for i in range(NQ):
                if i == 1:
                    # mask is only needed at attention; keep it behind
                    # quarter 0's weight loads on the scalar queue
                    nc.scalar.dma_start(msk, msk_in[:])
                # K heads 4i..4i+3 (weights balanced across both queues)
                kst = kst_pool.tile([P, HPQ, TOK], F8, tag="kst", name="kst")
                for l in range(HPQ):
                    h = i * HPQ + l
                    wksb = wqk_pool.tile([P, KD, HD], BF, tag="wqk",
                                         name="wksb")
                    (nc.sync if l % 2 == 0 else nc.scalar).dma_start(
                        wksb, wk_in[h])
                    psk = pp.tile([P, TOK], F32, tag="acc", name="psk")
                    for kt in range(KD):
                        nc.tensor.matmul(psk, lhsT=wksb[:, kt],
                                         rhs=n1T[:, kt],
                                         start=(kt == 0),
                                         stop=(kt == KD - 1))
                    nc.vector.tensor_copy(out=kst[:, l], in_=psk)
                nc.gpsimd.dma_start(kin_k[i], kst)

                # V output columns 512i..512(i+1) (heads 4i..4i+3)
                wvq = wv_pool.tile([P, KD, 512], BF, tag="wv", name="wvq")
                (nc.sync if i % 2 == 0 else nc.scalar).dma_start(
                    wvq, wv_in[i])
                vst = vst_pool.tile([P, NT, 512], F8, tag="vst", name="vst")
                for mt in range(NT):
                    psv = pp.tile([P, 512], F32, tag="acc", name="psv")
                    for kt in range(KD):
                        nc.tensor.matmul(
                            psv, lhsT=n1T[:, kt, mt * P:(mt + 1) * P],
                            rhs=wvq[:, kt],
                            start=(kt == 0), stop=(kt == KD - 1))
                    nc.vector.tensor_copy(out=vst[:, mt], in_=psv)
                nc.gpsimd.dma_start(kin_v[i], vst)

                # Q heads 4i..4i+3 (stays local)
                for l in range(HPQ):
                    h = i * HPQ + l
                    wqsb = wqk_pool.tile([P, KD, HD], BF, tag="wqk",
                                         name="wqsb")
                    (nc.scalar if l % 2 == 0 else nc.sync).dma_start(
                        wqsb, wq_in[h])
                    psq = pp.tile([P, TOK], F32, tag="acc", name="psq")
                    for kt in range(KD):
                        nc.tensor.matmul(psq, lhsT=wqsb[:, kt],
                                         rhs=n1T[:, kt],
                                         start=(kt == 0),
                                         stop=(kt == KD - 1))
                    nc.vector.tensor_copy(out=qt[:, h], in_=psq)

                nc.gpsimd.collective_compute(
                    "AllGather", mybir.AluOpType.bypass, replica_groups=RG,
                    ins=[kvin[i].opt()],
                    outs=[kvout[i].opt()],
                )
            # Gathered loads at the tail of the sync stream: every weight DMA
            # is already enqueued, and the sync engine has no further work
            # until the wo-phase weight loads (needed only after attention),
            # so stalling it on the AllGather semaphores delays nothing.
            for i in range(NQ):
                for r in range(GRP):
                    nc.sync.dma_start(katt[i][:, r], kout_k(i, r))
                    nc.sync.dma_start(vall[i][:, r], kout_v(i, r))

        # ---- phase 2: attention ----
        # Software-pipelined across heads: head h's 16 score matmuls stream
        # on the PE while the exp pipeline (scalar) fills head h's ex blocks
        # and the mask multiplies run block-batched on vector (blocks 0-2)
        # and gpsimd (block 3). The DoubleRow AV + denominator matmuls of
        # head h-1 interleave into the same PE stream; their inputs are a
        # full head behind, so they never stall. The denominator comes from
        # DoubleRow matmuls against all-ones directly on the fp8 ex blocks
        # (no vector reductions at all), and 1/dn runs on the scalar engine
        # as exp(-ln(dn)) — the DVE reciprocal on a 1-partition tile costs
        # 3.3us.
        with tc.tile_pool(name="sc_ps", bufs=2, space="PSUM") as sc_ps, \
             tc.tile_pool(name="av_ps", bufs=2, space="PSUM") as av_ps, \
             tc.tile_pool(name="dn_ps", bufs=2, space="PSUM") as dn_ps, \
             tc.tile_pool(name="exb_pool", bufs=12) as exb_pool, \
             tc.tile_pool(name="bi_pool", bufs=2) as bi_pool:
            state = {}          # per-head tiles for the h-1 pipeline

            def emit_av(h, u):
                """AV DoubleRow matmul for head h, kv-tile pair u. The av
                accumulator is allocated lazily at the first matmul so only
                the consuming head holds a PSUM bank."""
                i, l = h // HPQ, h % HPQ
                st = state[h]
                if "av" not in st:
                    st["av"] = av_ps.tile([P, C], F32, tag="av", name="av")
                r, m0 = (2 * u) // NT, (2 * u) % NT
                vsrc = vall[i][:, r, m0:m0 + 2, l * P:(l + 1) * P]
                nc.tensor.matmul(st["av"],
                                 lhsT=vsrc,
                                 rhs=st["exb"][u // 2][:, (2 * u) % 4:
                                                       (2 * u) % 4 + 2],
                                 perf_mode=DR,
                                 start=(u == 0), stop=(u == NKV // 2 - 1))

            def emit_dn(h, u):
                """Denominator DoubleRow matmul: ones8 x ex pair."""
                st = state[h]
                if "dn" not in st:
                    st["dn"] = dn_ps.tile([P, C], F32, tag="dn", name="dn")
                nc.tensor.matmul(st["dn"][0:1], lhsT=ones8[:, :, 0:1],
                                 rhs=st["exb"][u // 2][:, (2 * u) % 4:
                                                       (2 * u) % 4 + 2],
                                 perf_mode=DR,
                                 start=(u == 0), stop=(u == NKV // 2 - 1))

            def emit_finish(h):
                """1/dn (scalar ln/exp), rank-1 PE broadcast, avt write."""
                st = state.pop(h)
                lnd = bi_pool.tile([1, C], F32, tag="lnd", name="lnd")
                nc.scalar.activation(out=lnd, in_=st["dn"][0:1], func=AF.Ln)
                inv = bi_pool.tile([1, C], F32, tag="inv", name="inv")
                nc.scalar.activation(out=inv, in_=lnd, func=AF.Exp,
                                     scale=-1.0)
                bips = dn_ps.tile([P, C], F32, tag="dn", name="bips")
                nc.tensor.matmul(bips, lhsT=ones_row, rhs=inv,
                                 start=True, stop=True)
                bi = bi_pool.tile([P, C], F32, tag="bi", name="bi")
                nc.vector.tensor_copy(out=bi, in_=bips)
                nc.vector.tensor_tensor(
                    out=avt[:, h], in0=st["av"], in1=bi,
                    op=mybir.AluOpType.mult)

            def emit_head(h):
                """Scores+exp+masks of head h, AV/dn of head h-2. Two heads
                of lag means the AV/dn inputs are always long since ready,
                so the PE stream never stalls on the exp/mask pipeline."""
                i, l = h // HPQ, h % HPQ
                qv = qt[:, h]
                st = {
                    "exb": [exb_pool.tile([P, 4, C], F8E5, tag="exb",
                                          name="exb") for _ in range(4)],
                }
                state[h] = st
                for jj in range(NKV // 2):
                    b = jj // 2
                    sc = sc_ps.tile([P, 2, C], F32, tag="sc", name="sc")
                    for u in range(2):
                        j = 2 * jj + u
                        ksrc = katt[i][:, j // NT, l,
                                       (j % NT) * P:(j % NT) * P + P]
                        nc.tensor.matmul(sc[:, u], lhsT=ksrc, rhs=qv,
                                         start=True, stop=True)
                    # pair-batched exp straight into the ex block
                    nc.scalar.activation(
                        out=st["exb"][b][:, (2 * jj) % 4:(2 * jj) % 4 + 2],
                        in_=sc, func=AF.Exp, scale=INV_SQRT_HD)
                    if jj % 2 == 1:
                        # block-batched mask multiply (gpsimd is 4x slower
                        # than the DVE on this op; keep it all on vector)
                        nc.vector.tensor_mul(st["exb"][b], st["exb"][b],
                                             msk[:, 4 * b:4 * b + 4])
                    # head h-2's AV + dn matmuls
                    if h - 2 in state:
                        emit_av(h - 2, jj)
                        emit_dn(h - 2, jj)
                        if jj == NKV // 2 - 1:
                            emit_finish(h - 2)

            for h in range(H):
                emit_head(h)
            for h in (H - 2, H - 1):
                for u in range(NKV // 2):
                    emit_av(h, u)
                    emit_dn(h, u)
                emit_finish(h)
        kv_ctx.close()
        msk_ctx.close()
        xtp_ctx.close()

        # ---- phase 3: wo projection (fp8 DoubleRow) + residual ----
        x2p = ctx.enter_context(tc.tile_pool(name="x2p", bufs=1))
        x2 = x2p.tile([P, NT, D], F32, tag="x2", name="x2")

        stp = ctx.enter_context(tc.tile_pool(name="stp", bufs=1))
        stats4 = [stp.tile([P, NSUB, BN_STATS_DIM], F32, tag=f"st{mt}",
                           name=f"st{mt}") for mt in range(NT)]
        rinv4 = [stp.tile([P, 1], F32, tag=f"ri{mt}", name=f"ri{mt}")
                 for mt in range(NT)]
        assert NSUB == NC_

        def finish_rms(mt):
            mv = scratch.tile([P, BN_AGGR_DIM], F32, tag="bag", name="mv")
            nc.vector.bn_aggr(out=mv, in_=stats4[mt])
            msq = scratch.tile([P, 1], F32, tag="msq", name="msq")
            nc.vector.tensor_mul(msq, mv[:, 0:1], mv[:, 0:1])
            nc.vector.tensor_add(msq, msq, mv[:, 1:2])
            lnv = scratch.tile([P, 1], F32, tag="lnv", name="lnv")
            nc.scalar.activation(out=lnv, in_=msq, func=AF.Ln)
            rms = scratch.tile([P, 1], F32, tag="rms", name="rms")
            nc.scalar.activation(out=rms, in_=lnv, func=AF.Exp, scale=0.5)
            nc.vector.tensor_scalar_add(rms, rms, EPS)
            nc.vector.reciprocal(rinv4[mt], rms)

        # mt-outer so each token tile's rms stats + norm3 transposes overlap
        # the next tile's wo matmuls instead of serializing at the end
        n3T = persist.tile([P, KD, TOK], BF, tag="nT", name="nT")

        def norm3_transpose(mt, pp):
            nbf = scratch.tile([P, D], BF, tag="nbf", name="nbf")
            nc.vector.tensor_scalar_mul(nbf, x2[:, mt], rinv4[mt])
            transpose_into(n3T, nbf, mt, pp)

        with tc.tile_pool(name="wops", bufs=4, space="PSUM") as wops, \
             tc.tile_pool(name="wo_pool", bufs=4) as wo_pool:
            wons = []
            for n in range(NC_):
                won = wo_pool.tile([P, H // 2, 2, 512], F8, tag="wo",
                                   name="won")
                nc.sync.dma_start(won, wo_in[n])
                wons.append(won)
            for mt in range(NT):
                pss = [wops.tile([P, 512], F32, tag="acc", name=f"pso{n}")
                       for n in range(NC_)]
                for n in range(NC_):
                    for hh in range(H // 2):
                        nc.tensor.matmul(
                            pss[n],
                            lhsT=avt[:, 2 * hh:2 * hh + 2,
                                     mt * P:(mt + 1) * P],
                            rhs=wons[n][:, hh],
                            perf_mode=DR,
                            start=(hh == 0), stop=(hh == H // 2 - 1))
                for n in range(NC_):
                    xre = scratch.tile([P, 512], F32, tag="xre", name="xre")
                    nc.scalar.dma_start(xre, xv[:, mt, n * 512:(n + 1) * 512])
                    nc.vector.tensor_add(
                        out=x2[:, mt, n * 512:(n + 1) * 512],
                        in0=pss[n], in1=xre)
                    nc.vector.bn_stats(out=stats4[mt][:, n],
                                       in_=x2[:, mt, n * 512:(n + 1) * 512])
                finish_rms(mt)
                if mt > 0:
                    norm3_transpose(mt - 1, wops)
            norm3_transpose(NT - 1, wops)

        # ---- phase 4: norm3 + fc1 + fc2 (bf16) ----
        with tc.tile_pool(name="mm", bufs=4, space="PSUM") as mm, \
             tc.tile_pool(name="f1_pool", bufs=3) as f1_pool, \
             tc.tile_pool(name="f2_pool", bufs=3) as f2_pool, \
             tc.tile_pool(name="f2bp", bufs=1) as f2bp:
            f2b_bc = f2bp.tile([P, D], F32, tag="f2b", name="f2b")
            f2b_ap = f2b_in.ap()
            nc.gpsimd.dma_start(
                out=f2b_bc,
                in_=bass.AP(tensor=f2b_ap.tensor, offset=f2b_ap.offset,
                            ap=[[0, P], [1, D]]),
            )

            with tc.tile_pool(name="ht_pool", bufs=1) as ht_pool:
                hT = ht_pool.tile([P, NDF, TOK], BF, tag="hT", name="hT")
                for dt in range(NDF):
                    wsb = f1_pool.tile([P, KD, HD], BF, tag="f1", name="f1sb")
                    eng = nc.sync if dt % 2 == 0 else nc.scalar
                    eng.dma_start(wsb, f1_in[dt])
                    ps = mm.tile([P, TOK], F32, tag="acc", name="psf1")
                    for kt in range(KD):
                        nc.tensor.matmul(ps, lhsT=wsb[:, kt], rhs=n3T[:, kt],
                                         start=(kt == 0), stop=(kt == KD - 1))
                    nc.scalar.activation(out=hT[:, dt], in_=ps, func=AF.Silu,
                                         bias=f1b_sb[:, dt:dt + 1], scale=1.0)

                # fc2 + bias + residual -> y
                for n in range(NC_):
                    pss = [mm.tile([P, 512], F32, tag="acc", name=f"psf2{mt}")
                           for mt in range(NT)]
                    for oh in range(8):
                        wsb = f2_pool.tile([P, 8, 512], BF, tag="f2",
                                           name="f2sb")
                        eng = nc.sync if oh % 2 == 0 else nc.scalar
                        osub = (oh % 2) * 8
                        eng.dma_start(wsb,
                                      f2_in[n, oh // 2][:, osub:osub + 8])
                        for o in range(8):
                            dt = oh * 8 + o
                            for mt in range(NT):
                                nc.tensor.matmul(
                                    pss[mt],
                                    lhsT=hT[:, dt, mt * P:(mt + 1) * P],
                                    rhs=wsb[:, o],
                                    start=(dt == 0), stop=(dt == NDF - 1))
                    for mt in range(NT):
                        osb = scratch.tile([P, 512], F32, tag="osb", name="osb")
                        nc.vector.tensor_add(osb, pss[mt],
                                             x2[:, mt, n * 512:(n + 1) * 512])
                        nc.vector.tensor_add(osb, osb,
                                             f2b_bc[:, n * 512:(n + 1) * 512])
                        nc.gpsimd.dma_start(yv[:, mt, n * 512:(n + 1) * 512],
                                            osb)


def build_program():
    nc = bacc.Bacc("TRN2", target_bir_lowering=False, debug=False,
                   num_devices=N_CORES)
    _emit(nc)
    nc.finalize()
    return nc


def _bf(x):
    return np.ascontiguousarray(x.astype(ml_dtypes.bfloat16))


def _f8(x):
    return np.ascontiguousarray(
        np.clip(x, -240.0, 240.0).astype(ml_dtypes.float8_e4m3fn))


def prep_inputs(inputs):
    """Host-side prep: fold alpha into weights, build per-core in_maps."""
    x = np.asarray(inputs["x"], dtype=np.float32)
    tgt = np.asarray(inputs["tgt_mask"])
    wq = np.asarray(inputs["wq"], dtype=np.float32)
    wk = np.asarray(inputs["wk"], dtype=np.float32)
    wv = np.asarray(inputs["wv"], dtype=np.float32)
    wo = np.asarray(inputs["wo"], dtype=np.float32)
    a1 = np.asarray(inputs["alpha1"], dtype=np.float32)
    a3 = np.asarray(inputs["alpha3"], dtype=np.float32)
    f1w = np.asarray(inputs["fc1_w"], dtype=np.float32)
    f1b = np.asarray(inputs["fc1_b"], dtype=np.float32)
    f2w = np.asarray(inputs["fc2_w"], dtype=np.float32)
    f2b = np.asarray(inputs["fc2_b"], dtype=np.float32)

    wqT = (wq * a1[None, :]).T                           # [D_in, D_out]
    wkT = (wk * a1[None, :]).T
    wvT = (wv * a1[None, :]).T
    woT = wo.T
    f1T = (f1w * a3[None, :]).T                          # [D, DFF]
    f2T = f2w.T                                          # [DFF, D]

    # stationary pre-tiling: [out_tile, partition(k), k_tile, out_sub]
    wqt = _bf(wqT.reshape(KD, P, H, HD).transpose(2, 1, 0, 3))
    wkt = _bf(wkT.reshape(KD, P, H, HD).transpose(2, 1, 0, 3))
    f1t = _bf(f1T.reshape(KD, P, NDF, HD).transpose(2, 1, 0, 3))
    # moving-weight pre-tiling, contiguous per partition line:
    wvt = _bf(wvT.reshape(KD, P, NQ, 512).transpose(2, 1, 0, 3))  # [NQ,P,KD,512]
    # wo in fp8 DoubleRow layout: [n, p, hh, i, c] = woT[(2hh+i)*128+p, n*512+c]
    wot = _f8(woT.reshape(H // 2, 2, P, NC_, 512).transpose(3, 2, 0, 1, 4))
    f2t = _bf(f2T.reshape(4, 16, P, NC_, 512).transpose(3, 0, 2, 1, 4))
    f1b_t = np.ascontiguousarray(f1b.reshape(NDF, P).T.astype(np.float32))

    tm = np.asarray(tgt[0, 0], dtype=np.float32)         # [S, S]
    in_maps = []
    for c in range(N_CORES):
        b = c // GRP
        rows = slice((c % GRP) * C, (c % GRP + 1) * C)
        x_own = np.ascontiguousarray(x[b, rows])
        mask = np.ascontiguousarray(
            tm[rows, :].T.reshape(NKV, P, C).transpose(1, 0, 2)
            .astype(ml_dtypes.float8_e5m2))
        in_maps.append({
            "x_own": x_own,
            "wqt": wqt, "wkt": wkt, "wvt": wvt, "wot": wot,
            "fc1t": f1t, "fc2t": f2t,
            "fc1b": f1b_t, "fc2b": f2b,
            "mask": mask,
        })
    return in_maps


def assemble_output(results):
    y = np.empty((B, S, D), dtype=np.float32)
    for c in range(N_CORES):
        yc = results[c]["y"]                              # [TOK, D]
        y[c // GRP, (c % GRP) * C:(c % GRP + 1) * C] = yc
    return y


_CACHE = {}


def kernel(**inputs) -> np.ndarray:
    if "nc" not in _CACHE:
        _CACHE["nc"] = build_program()
    nc = _CACHE["nc"]
    in_maps = prep_inputs(inputs)
    res = run_bass_kernel_spmd(nc, in_maps, core_ids=list(range(N_CORES)))
    return assemble_output(res.results)
